# revision 23
# baseline (speedup 1.0000x reference)
"""Cascaded-attention GRU recurrence on 8 NeuronCores (Bass/Tile), v3.

Problem: B=128, T=75, D=512, V=28. Data-parallel over batch: 16 batch rows
per core, weights replicated. Per-core recurrence over 75 steps with two
INDEPENDENT batch half-groups (8 rows each) running half a step out of
phase so the Activation engine (the per-step floor: tanh over B*T*D) stays
saturated while the other group walks its gate/state-update chain.

Per-core layout (BL=16 local batch, G=2 groups of BG=8):
- UaH' = x@Ua + Ba1 + Ba2 stored bf16 [128(d), 4(chunk), 75(t), 16(b)]
  (t-before-b: the per-step bias broadcast add has stride-1 last dim ->
  DVE 2x mode).
- per step, per group: WaS^T = Wa^T stateT (PE fp32 psum) -> bf16 SBUF;
  Y = UaH' + WaS'(bcast over t), b-half on DVE (bf16 2x) / b-half GPSIMD;
  tanh split in two b-half ACT instrs so the attention matmuls for the
  first b-half start at the half-way point.
- scores vs column-replicated bf16 Va (PE 1cyc/row) -> REP[75,(b,t)] psum;
  diag-mask (DVE) + X-reduce gives scoresT[t, b]; exp (ACT) writes onto
  the block-diagonal of smb[75, 72] (stride-9 AP; zeros persist from
  preamble).
- xm/CoC/Z: 8 accumulated matmuls smb-window^T @ xkc[t,b,113] (bf16);
  col 112 of xkc is ones so xm[:,112] = Z.
- GRU gates: sigmoid via tanh (rec kernel pre-halved on host, 1/(2Z)
  folded into gate scalars). State update computed directly TRANSPOSED:
  nsT = (0.5+0.5 uzT) * sT + (0.5-0.5 uzT) * hhT with uzT/hhT via PE
  transposes, written straight into stateT -- no b-major state tensor.
- output softmax fp32; embedding lookup exact via is_ge (probs cast to
  int32 are 0 unless pred >= 1.0); gru_bias[0] assumed zero (asserted).
"""

import numpy as np

B, T, D, V = 128, 75, 512, 28
NCORES = 8
BL = B // NCORES
G = 2
BG = BL // G
NC_, CH = 128, D // 128
BH = BG // 2            # b-half within a group (tanh/bias/score split)


def _build(nc, tc, tile, bass, mybir, gru_b0_nonzero, steps=T):
    f32 = mybir.dt.float32
    bf16 = mybir.dt.bfloat16
    Act = mybir.ActivationFunctionType
    Op = mybir.AluOpType
    AP = bass.AP

    dr = {}
    def din(name, shape, dt=f32):
        dr[name] = nc.dram_tensor(name, shape, dt, kind="ExternalInput")
        return dr[name]

    x_b = din("x_b", [NC_, CH, BL, T], bf16)
    ua_k = din("ua_k", [NC_, CH, CH, 128], bf16)
    ba12 = din("ba12", [NC_, CH])
    wa = din("wa", [V, D])
    va = din("va", [NC_, CH], bf16)
    w2 = din("w2", [NC_, CH, 112], bf16)
    wrec_h = din("wrec_h", [V + 1, 84])
    uo = din("uo", [V + 1, V])
    i8 = din("i8", [BG, BG])
    onesrow = din("onesrow", [1, BL])
    dwrep = din("dwrep", [BG, V])
    if gru_b0_nonzero:
        b0rep = din("b0rep", [BG, 84])
    y_out = nc.dram_tensor("y", [BL, T, V], f32, kind="ExternalOutput")

    import contextlib
    ctx = contextlib.ExitStack()
    with ctx:
        cst = ctx.enter_context(tc.tile_pool(name="cst", bufs=1))
        wrk = ctx.enter_context(tc.tile_pool(name="wrk", bufs=2))
        pwast = ctx.enter_context(tc.tile_pool(name="pwast", bufs=1, space="PSUM"))
        prep = ctx.enter_context(tc.tile_pool(name="prep", bufs=2, space="PSUM"))
        pxm = ctx.enter_context(tc.tile_pool(name="pxm", bufs=3, space="PSUM"))
        psc = ctx.enter_context(tc.tile_pool(name="psc", bufs=2, space="PSUM"))

        # ---------------- constants ----------------
        t_x = cst.tile([NC_, CH, BL, T], bf16, tag="t_x")
        t_ua = cst.tile([NC_, CH, CH, 128], bf16, tag="t_ua")
        t_ba12 = cst.tile([NC_, CH], f32, tag="t_ba12")
        t_wa = cst.tile([V, D], f32, tag="t_wa")
        t_va = cst.tile([NC_, CH], bf16, tag="t_va")
        t_w2 = cst.tile([NC_, CH, 112], bf16, tag="t_w2")
        t_wrec = cst.tile([V + 1, 84], f32, tag="t_wrec")
        t_uo = cst.tile([V + 1, V], f32, tag="t_uo")
        t_i8 = cst.tile([BG, BG], f32, tag="t_i8")
        t_dw = cst.tile([BG, V], f32, tag="t_dw")
        t_onesT = cst.tile([V, BG], f32, tag="t_onesT")
        nc.vector.memset(t_onesT[:], 1.0)
        for tt, d_ in [(t_x, x_b), (t_ua, ua_k), (t_ba12, ba12), (t_wa, wa),
                       (t_va, va), (t_w2, w2), (t_wrec, wrec_h),
                       (t_uo, uo), (t_i8, i8), (t_dw, dwrep)]:
            nc.sync.dma_start(tt[:], d_[:])
        if gru_b0_nonzero:
            t_b0h = cst.tile([BG, 84], f32, tag="t_b0")
            nc.sync.dma_start(t_b0h[:], b0rep[:])

        t_uahp = cst.tile([NC_, CH, T, BL], bf16, tag="t_uahp")
        t_xkc = cst.tile([T, BL, 113], bf16, tag="t_xkc")
        t_tanh = [cst.tile([NC_, CH, T, BG], bf16, tag=f"t_tanh{g}",
                           name=f"t_tanh{g}") for g in range(G)]
        t_smb = [cst.tile([T, BG * (BG + 1)], bf16, tag=f"t_smb{g}",
                          name=f"t_smb{g}") for g in range(G)]
        t_wasb = [cst.tile([NC_, CH, BG], bf16, tag=f"t_wasb{g}",
                           name=f"t_wasb{g}") for g in range(G)]
        t_stT = [cst.tile([V + 1, BG], f32, tag=f"t_stT{g}",
                          name=f"t_stT{g}") for g in range(G)]
        t_out = []
        for g in range(G):
            t_out_g = cst.tile([BG, T, V], f32, tag=f"t_out{g}")
            if steps < T:
                nc.vector.memset(t_out_g[:], 0.0)
            t_out.append(t_out_g)
            nc.vector.memset(t_smb[g][:], 0.0)
            nc.vector.memset(t_stT[g][0:V, :], 0.0)
            nc.sync.dma_start(t_stT[g][V:V + 1, :], onesrow[:, g * BG:(g + 1) * BG])

        # ---------------- preamble: UaH' ----------------
        NSL, SLB = 4, 4
        for ec in range(CH):
            for i in range(NSL):
                b0 = i * SLB
                ps = prep.tile([NC_, T * SLB], f32, tag="rep",
                               name=f"preu{ec}_{i}")
                psv = ps[:, 0:T * SLB].rearrange("p (t b) -> p t b", t=T)
                for dc in range(CH):
                    xs = t_x[:, dc, :, :]
                    x_sl = AP(xs.tensor, xs.offset + b0 * T,
                              [list(xs.ap[0]), [1, T], [T, SLB]])
                    nc.tensor.matmul(psv, t_ua[:, dc, ec, :], x_sl,
                                     start=(dc == 0), stop=(dc == CH - 1))
                dst = t_uahp[:, ec, :, b0:b0 + SLB]
                if i % 2 == 0:
                    nc.scalar.activation(dst, psv, Act.Identity,
                                         bias=t_ba12[:, ec:ec + 1], scale=1.0)
                else:
                    nc.vector.tensor_scalar(dst, psv, t_ba12[:, ec:ec + 1],
                                            None, Op.add)

        # ---------------- preamble: XKC ----------------
        for b in range(BL):
            ps = pxm.tile([T, 113], f32, tag="xmbt", name=f"prex{b}")
            for dc in range(CH):
                nc.tensor.matmul(ps[:, 0:112], t_x[:, dc, b, :], t_w2[:, dc, :],
                                 start=(dc == 0), stop=(dc == CH - 1))
            if b % 2 == 0:
                nc.scalar.activation(t_xkc[:, b, 0:112], ps[:, 0:112],
                                     Act.Identity)
            else:
                nc.vector.tensor_copy(t_xkc[:, b, 0:112], ps[:, 0:112])
        ones_col = AP(t_xkc.tensor, t_xkc[:].offset + 112,
                      [list(t_xkc[:].ap[0]), [113, BL]])
        nc.vector.memset(ones_col, 1.0)

        P = {}

        def emit_bt(g, s):
            """hm/uo matmuls into the xmbt psum tile; reads stateT(s).
            hm is copied to SBUF (DVE may read only one PSUM input, and zr
            already reads xm from PSUM)."""
            xmbt = pxm.tile([BG, 240], f32, tag="xmbt", name=f"xmbt{g}_{s}")
            P[(g, s, "xmbt")] = xmbt
            nc.tensor.matmul(xmbt[:, 128:212], t_stT[g][:], t_wrec[:],
                             start=True, stop=True)
            nc.tensor.matmul(xmbt[:, 212:240], t_stT[g][:], t_uo[:],
                             start=True, stop=True)
            hm = wrk.tile([BG, 84], f32, tag=f"hm{g}", name=f"hm{g}_{s}")
            nc.vector.tensor_copy(hm[:], xmbt[:, 128:212])
            P[(g, s, "hm")] = hm

        def phase_TH(g, s, half):
            b0 = half * BH
            if s == 0:
                src_ = t_uahp[:, :, :, g * BG + b0:g * BG + b0 + BH]
            else:
                src_ = P[(g, s, "Y")][:, :, :, b0:b0 + BH]
            nc.scalar.activation(t_tanh[g][:, :, :, b0:b0 + BH], src_, Act.Tanh)

        def phase_SC1(g, s):
            """scoresT columns via per-b stationary matmuls (after tanhs)."""
            tg = t_tanh[g]
            scp = psc.tile([T, BG], f32, tag="scp", name=f"scp{g}_{s}")
            P[(g, s, "scp")] = scp
            for b in range(BG):
                for c in range(CH):
                    tc_ = tg[:, c, :, :]
                    stat = AP(tc_.tensor, tc_.offset + b,
                              [list(tc_.ap[0]), [BG, T]])
                    nc.tensor.matmul(scp[:, b:b + 1], stat,
                                     t_va[:, c:c + 1],
                                     start=(c == 0), stop=(c == CH - 1))

        def phase_SC2(g, s):
            """exp -> smb diag; xm accumulation; rZ; xh_n precompute."""
            scp = P.pop((g, s, "scp"))
            smb = t_smb[g]
            smb_diag = AP(smb.tensor, smb[:].offset,
                          [list(smb[:].ap[0]), [BG + 1, BG]])
            nc.scalar.activation(smb_diag, scp[:], Act.Exp)
            xm = P[(g, s, "xmbt")][:, 0:113]
            for b in range(BG):
                nc.tensor.matmul(xm, smb[:, BG * b:BG * b + BG],
                                 t_xkc[:, g * BG + b, :],
                                 start=(b == 0), stop=(b == BG - 1))
            rZ = wrk.tile([BG, 1], f32, tag=f"rZ{g}", name=f"rZ{g}_{s}")
            nc.vector.reciprocal(rZ[:], xm[:, 112:113])
            P[(g, s, "rZ")] = rZ
            xh = wrk.tile([BG, V], f32, tag=f"xh{g}", name=f"xh{g}_{s}")
            nc.vector.tensor_scalar(xh[:], xm[:, 56:84], rZ[:], None, Op.mult)
            P[(g, s, "xh")] = xh

        def phase_G1(g, s):
            """zr -> tz; transpose; z1/w1/m1 (off hh-chain)."""
            xmbt = P[(g, s, "xmbt")]
            zr = wrk.tile([BG, 56], f32, tag=f"zr{g}", name=f"zr{g}_{s}")
            nc.vector.scalar_tensor_tensor(zr[:], xmbt[:, 0:56],
                                           P[(g, s, "rZ")][:],
                                           P[(g, s, "hm")][:, 0:56],
                                           Op.mult, Op.add)
            if gru_b0_nonzero:
                nc.vector.tensor_tensor(zr[:], zr[:], t_b0h[:, 0:56], Op.add)
            tz = wrk.tile([BG, 56], f32, tag=f"tz{g}", name=f"tz{g}_{s}")
            nc.scalar.activation(tz[:], zr[:], Act.Tanh)
            P[(g, s, "tz")] = tz
            wast = pwast.tile([NC_, 48], f32, tag="wast", name=f"wast{g}_{s}")
            P[(g, s, "wast")] = wast
            nc.tensor.transpose(wast[0:V, 32:40], tz[:, 0:V], t_i8[:])
            z1 = wrk.tile([V, BG], f32, tag=f"z1{g}", name=f"z1{g}_{s}")
            nc.vector.tensor_scalar(z1[:], wast[0:V, 32:40], 0.5, 0.5,
                                    Op.mult, Op.add)
            w1 = wrk.tile([V, BG], f32, tag=f"w1{g}", name=f"w1{g}_{s}")
            nc.gpsimd.tensor_tensor(w1[:], t_onesT[:], z1[:], Op.subtract)
            m1 = wrk.tile([V, BG], f32, tag=f"m1{g}", name=f"m1{g}_{s}")
            nc.gpsimd.tensor_mul(m1[:], z1[:], t_stT[g][0:V, :])
            P[(g, s, "w1")] = w1
            P[(g, s, "m1")] = m1

        def phase_E1(g, s):
            """output logits + expP (slots between tz_z and hh on ACT)."""
            xmbt = P[(g, s, "xmbt")]
            rZ = P[(g, s, "rZ")]
            l1 = wrk.tile([BG, V], f32, tag=f"l1{g}", name=f"l1{g}_{s}")
            if s > 0:
                l2 = wrk.tile([BG, V], f32, tag=f"l2{g}", name=f"l2{g}_{s}")
                nc.vector.scalar_tensor_tensor(l2[:], t_out[g][:, s - 1, :],
                                               1.0, t_dw[:], Op.is_ge,
                                               Op.mult)
                nc.vector.scalar_tensor_tensor(l1[:], xmbt[:, 84:112], rZ[:],
                                               l2[:], Op.mult, Op.add)
            else:
                nc.vector.tensor_scalar(l1[:], xmbt[:, 84:112], rZ[:], None,
                                        Op.mult)
            logits = wrk.tile([BG, V], f32, tag=f"lg{g}", name=f"lg{g}_{s}")
            nc.vector.tensor_tensor(logits[:], l1[:], xmbt[:, 212:240], Op.add)
            expP = wrk.tile([BG, V], f32, tag=f"eP{g}", name=f"eP{g}_{s}")
            nc.scalar.activation(expP[:], logits[:], Act.Exp)
            P[(g, s, "expP")] = expP

        def phase_G2a(g, s):
            """s1 -> ah -> hh (the on-chain ACT op)."""
            xmbt = P[(g, s, "xmbt")]
            tz = P[(g, s, "tz")]
            s1 = wrk.tile([BG, V], f32, tag=f"s1{g}", name=f"s1{g}_{s}")
            nc.vector.scalar_tensor_tensor(s1[:], tz[:, V:56], 1.0,
                                           P[(g, s, "hm")][:, 56:84],
                                           Op.add, Op.mult)
            ah = wrk.tile([BG, V], f32, tag=f"ah{g}", name=f"ah{g}_{s}")
            nc.vector.tensor_tensor(ah[:], P[(g, s, "xh")][:], s1[:], Op.add)
            if gru_b0_nonzero:
                nc.vector.tensor_tensor(ah[:], ah[:], t_b0h[:, 56:84], Op.add)
            hh = wrk.tile([BG, V], f32, tag=f"hh{g}", name=f"hh{g}_{s}")
            nc.scalar.activation(hh[:], ah[:], Act.Tanh)
            P[(g, s, "hh")] = hh

        def phase_G2b(g, s):
            """state update + WaS/bias for s+1 (hidden under A's tanh2)."""
            wast = P[(g, s, "wast")]
            hh = P[(g, s, "hh")]
            nc.tensor.transpose(wast[0:V, 40:48], hh[:], t_i8[:])
            n1 = wrk.tile([V, BG], f32, tag=f"n1{g}", name=f"n1{g}_{s}")
            nc.vector.tensor_tensor(n1[:], P[(g, s, "w1")][:],
                                    wast[0:V, 40:48], Op.mult)
            nc.vector.tensor_tensor(t_stT[g][0:V, :], n1[:],
                                    P[(g, s, "m1")][:], Op.add)
            if s + 1 >= steps:
                return
            for c in range(CH):
                nc.tensor.matmul(wast[:, c * BG:(c + 1) * BG],
                                 t_wa[:, c * 128:(c + 1) * 128],
                                 t_stT[g][0:V, :], start=True, stop=True)
            emit_bt(g, s + 1)
            wb = t_wasb[g]
            nc.vector.tensor_copy(wb[:].rearrange("p c b -> p (c b)"),
                                  wast[:, 0:CH * BG])
            ty = P[(g, s + 1, "Y")] = wrk.tile([NC_, CH, T, BG], bf16,
                                               tag=f"Y{g}", name=f"Y{g}_{s+1}")
            uah_g = t_uahp[:, :, :, g * BG:(g + 1) * BG]
            def bias(eng, b0, bn):
                w_sl = AP(wb.tensor, wb[:].offset + b0,
                          [list(wb[:].ap[0]), [BG, CH], [0, T], [1, bn]])
                eng.tensor_tensor(ty[:, :, :, b0:b0 + bn],
                                  uah_g[:, :, :, b0:b0 + bn], w_sl, Op.add)
            bias(nc.vector, 0, BH)
            bias(nc.gpsimd, BH, BH)

        def phase_E2(g, s):
            """output softmax normalization (fully off-chain)."""
            xmbt = P.pop((g, s, "xmbt"))
            expP = P.pop((g, s, "expP"))
            for k in ("xh", "tz", "Y", "wast", "w1", "m1", "hh", "rZ", "hm"):
                P.pop((g, s, k), None)
            zp = wrk.tile([BG, 1], f32, tag=f"zp{g}", name=f"zp{g}_{s}")
            nc.vector.tensor_reduce(zp[:], expP[:], mybir.AxisListType.X,
                                    Op.add)
            rp = wrk.tile([BG, 1], f32, tag=f"rp{g}", name=f"rp{g}_{s}")
            nc.vector.reciprocal(rp[:], zp[:])
            rp_bc = AP(rp.tensor, rp[:].offset,
                       [list(rp[:].ap[0]), [0, V]])
            nc.gpsimd.tensor_tensor(t_out[g][:, s, :], expP[:], rp_bc,
                                    Op.mult)

        emit_bt(0, 0)
        emit_bt(1, 0)

        # ---- steps: per half-period, A tanh-ing step s, B walking its
        # attention/gates for the step whose tanhs ran last half ----
        def half(A, s_a, Bs):
            if Bs is not None:
                Bg, s_b = Bs
                phase_SC2(Bg, s_b)
            phase_TH(A, s_a, 0)
            if Bs is not None:
                phase_G1(Bg, s_b)
                phase_G2a(Bg, s_b)
                phase_G2b(Bg, s_b)
                phase_E1(Bg, s_b)
            phase_TH(A, s_a, 1)
            phase_SC1(A, s_a)
            if Bs is not None:
                phase_E2(Bg, s_b)

        for s in range(steps):
            half(0, s, (1, s - 1) if s > 0 else None)
            half(1, s, (0, s))
        phase_SC2(1, steps - 1)
        phase_G1(1, steps - 1)
        phase_E1(1, steps - 1)
        phase_E2(1, steps - 1)

        for g in range(G):
            nc.sync.dma_start(y_out[g * BG:(g + 1) * BG, :, :], t_out[g][:])
    return dr, y_out


_CACHE = {}


def _get_program(gru_b0_nonzero, steps=T):
    key = (bool(gru_b0_nonzero), steps)
    if key in _CACHE:
        return _CACHE[key]
    import concourse.bass as bass
    import concourse.bacc as bacc
    import concourse.tile as tile
    from concourse import mybir

    nc = bacc.Bacc("TRN2", target_bir_lowering=False, debug=False,
                   num_devices=NCORES)
    with tile.TileContext(nc) as tc:
        _build(nc, tc, tile, bass, mybir, gru_b0_nonzero, steps)
    nc.compile()
    _CACHE[key] = nc
    return nc


def _prep_core_inputs(inputs, core):
    import ml_dtypes
    x = inputs["x"]
    xs = np.ascontiguousarray(x[core * BL:(core + 1) * BL]).astype(np.float32)
    x_dmaj = np.ascontiguousarray(
        xs.reshape(BL, T, CH, 128).transpose(3, 2, 0, 1))
    return x_dmaj.astype(ml_dtypes.bfloat16)


def _prep_weights(inputs):
    import ml_dtypes
    f = np.float32
    bfd = ml_dtypes.bfloat16
    Ua = inputs["Ua"].astype(f)
    ua_k = np.ascontiguousarray(
        Ua.reshape(CH, 128, CH, 128).transpose(1, 0, 2, 3))
    ba = (inputs["Ba1"] + inputs["Ba2"]).astype(f).reshape(CH, 128)
    ba12 = np.ascontiguousarray(ba.T)
    Va = inputs["Va"].astype(f).reshape(CH, 128)
    va_t = np.ascontiguousarray(Va.T)
    gk = inputs["gru_kernel"].astype(f).copy()
    gk[:, 0:2 * V] *= 0.5
    w2 = np.concatenate([gk, inputs["Co"]], axis=1).astype(f)
    w2 = np.ascontiguousarray(w2.reshape(CH, 128, 112).transpose(1, 0, 2))
    w = (inputs["emb"].astype(f) @ inputs["Wo"].astype(f)).reshape(-1)
    w0, w1 = float(w[0]), float(w[1])
    gb = inputs["gru_bias"].astype(f)
    out = {
        "ua_k": ua_k.astype(bfd), "ba12": ba12,
        "wa": inputs["Wa"].astype(f),
        "va": va_t.astype(bfd), "w2": w2.astype(bfd),
        "wrec_h": np.concatenate(
            [0.5 * inputs["gru_rec_kernel"].astype(f), 0.5 * gb[1:2]], axis=0),
        "uo": np.concatenate(
            [inputs["Uo"].astype(f), inputs["Bo"].astype(f) + w0], axis=0),
        "i8": np.eye(BG, dtype=f),
        "onesrow": np.ones([1, BL], dtype=f),
        "dwrep": np.full([BG, V], w1 - w0, dtype=f),
    }
    b0 = gb[0].copy()
    b0[0:2 * V] *= 0.5
    if np.any(b0 != 0):
        out["b0rep"] = np.repeat(b0[None, :], BG, axis=0)
    return out, bool(np.any(b0 != 0))


def kernel(**inputs):
    from concourse.bass_utils import run_bass_kernel_spmd

    weights, b0nz = _prep_weights(inputs)
    nc = _get_program(b0nz)
    in_maps = []
    for core in range(NCORES):
        m = dict(weights)
        m["x_b"] = _prep_core_inputs(inputs, core)
        in_maps.append(m)
    res = run_bass_kernel_spmd(nc, in_maps, core_ids=list(range(NCORES)))
    out = np.concatenate([res.results[c]["y"] for c in range(NCORES)], axis=0)
    return out.astype(np.float32)


# revision 24
# speedup vs baseline: 2.1096x; 2.1096x over previous
"""Cascaded-attention GRU recurrence on 8 NeuronCores (Bass/Tile), v3.

Problem: B=128, T=75, D=512, V=28. Data-parallel over batch: 16 batch rows
per core, weights replicated. Per-core recurrence over 75 steps with two
INDEPENDENT batch half-groups (8 rows each) running half a step out of
phase so the Activation engine (the per-step floor: tanh over B*T*D) stays
saturated while the other group walks its gate/state-update chain.

Per-core layout (BL=16 local batch, G=2 groups of BG=8):
- UaH' = x@Ua + Ba1 + Ba2 stored bf16 [128(d), 4(chunk), 75(t), 16(b)]
  (t-before-b: the per-step bias broadcast add has stride-1 last dim ->
  DVE 2x mode).
- per step, per group: WaS^T = Wa^T stateT (PE fp32 psum) -> bf16 SBUF;
  Y = UaH' + WaS'(bcast over t), b-half on DVE (bf16 2x) / b-half GPSIMD;
  tanh split in two b-half ACT instrs so the attention matmuls for the
  first b-half start at the half-way point.
- scores vs column-replicated bf16 Va (PE 1cyc/row) -> REP[75,(b,t)] psum;
  diag-mask (DVE) + X-reduce gives scoresT[t, b]; exp (ACT) writes onto
  the block-diagonal of smb[75, 72] (stride-9 AP; zeros persist from
  preamble).
- xm/CoC/Z: 8 accumulated matmuls smb-window^T @ xkc[t,b,113] (bf16);
  col 112 of xkc is ones so xm[:,112] = Z.
- GRU gates: sigmoid via tanh (rec kernel pre-halved on host, 1/(2Z)
  folded into gate scalars). State update computed directly TRANSPOSED:
  nsT = (0.5+0.5 uzT) * sT + (0.5-0.5 uzT) * hhT with uzT/hhT via PE
  transposes, written straight into stateT -- no b-major state tensor.
- output softmax fp32; embedding lookup exact via is_ge (probs cast to
  int32 are 0 unless pred >= 1.0); gru_bias[0] assumed zero (asserted).
"""

import numpy as np

B, T, D, V = 128, 75, 512, 28
NCORES = 8
BL = B // NCORES
G = 2
BG = BL // G
NC_, CH = 128, D // 128
BH = BG // 2            # b-half within a group (tanh/bias/score split)


def _build(nc, tc, tile, bass, mybir, gru_b0_nonzero, steps=T):
    f32 = mybir.dt.float32
    bf16 = mybir.dt.bfloat16
    Act = mybir.ActivationFunctionType
    Op = mybir.AluOpType
    AP = bass.AP

    dr = {}
    def din(name, shape, dt=f32):
        dr[name] = nc.dram_tensor(name, shape, dt, kind="ExternalInput")
        return dr[name]

    x_b = din("x_b", [NC_, CH, BL, T], bf16)
    ua_k = din("ua_k", [NC_, CH, CH, 128], bf16)
    ba12 = din("ba12", [NC_, CH])
    wa = din("wa", [V, D])
    va = din("va", [NC_, CH], bf16)
    w2 = din("w2", [NC_, CH, 112], bf16)
    wrec_h = din("wrec_h", [V + 1, 84])
    uo = din("uo", [V + 1, V])
    i8 = din("i8", [BG, BG])
    onesrow = din("onesrow", [1, BL])
    dwrep = din("dwrep", [BG, V])
    if gru_b0_nonzero:
        b0rep = din("b0rep", [BG, 84])
    y_out = nc.dram_tensor("y", [BL, T, V], f32, kind="ExternalOutput")

    import contextlib
    ctx = contextlib.ExitStack()
    with ctx:
        cst = ctx.enter_context(tc.tile_pool(name="cst", bufs=1))
        wrk = ctx.enter_context(tc.tile_pool(name="wrk", bufs=2))
        pwast = ctx.enter_context(tc.tile_pool(name="pwast", bufs=1, space="PSUM"))
        prep = ctx.enter_context(tc.tile_pool(name="prep", bufs=2, space="PSUM"))
        pxm = ctx.enter_context(tc.tile_pool(name="pxm", bufs=3, space="PSUM"))
        psc = ctx.enter_context(tc.tile_pool(name="psc", bufs=2, space="PSUM"))

        # ---------------- constants ----------------
        t_x = cst.tile([NC_, CH, BL, T], bf16, tag="t_x")
        t_ua = cst.tile([NC_, CH, CH, 128], bf16, tag="t_ua")
        t_ba12 = cst.tile([NC_, CH], f32, tag="t_ba12")
        t_wa = cst.tile([V, D], f32, tag="t_wa")
        t_va = cst.tile([NC_, CH], bf16, tag="t_va")
        t_w2 = cst.tile([NC_, CH, 112], bf16, tag="t_w2")
        t_wrec = cst.tile([V + 1, 84], f32, tag="t_wrec")
        t_uo = cst.tile([V + 1, V], f32, tag="t_uo")
        t_i8 = cst.tile([BG, BG], f32, tag="t_i8")
        t_dw = cst.tile([BG, V], f32, tag="t_dw")
        t_onesT = cst.tile([V, BG], f32, tag="t_onesT")
        nc.vector.memset(t_onesT[:], 1.0)
        for tt, d_ in [(t_x, x_b), (t_ua, ua_k), (t_ba12, ba12), (t_wa, wa),
                       (t_va, va), (t_w2, w2), (t_wrec, wrec_h),
                       (t_uo, uo), (t_i8, i8), (t_dw, dwrep)]:
            nc.sync.dma_start(tt[:], d_[:])
        if gru_b0_nonzero:
            t_b0h = cst.tile([BG, 84], f32, tag="t_b0")
            nc.sync.dma_start(t_b0h[:], b0rep[:])

        t_uahp = cst.tile([NC_, CH, T, BL], bf16, tag="t_uahp")
        t_xkc = cst.tile([T, BL, 113], bf16, tag="t_xkc")
        t_tanh = [cst.tile([NC_, CH, T, BG], bf16, tag=f"t_tanh{g}",
                           name=f"t_tanh{g}") for g in range(G)]
        t_smb = [cst.tile([T, BG * (BG + 1)], bf16, tag=f"t_smb{g}",
                          name=f"t_smb{g}") for g in range(G)]
        t_wasb = [cst.tile([NC_, CH, BG], bf16, tag=f"t_wasb{g}",
                           name=f"t_wasb{g}") for g in range(G)]
        t_stT = [cst.tile([V + 1, BG], f32, tag=f"t_stT{g}",
                          name=f"t_stT{g}") for g in range(G)]
        t_out = []
        for g in range(G):
            t_out_g = cst.tile([BG, T, V], f32, tag=f"t_out{g}")
            if steps < T:
                nc.vector.memset(t_out_g[:], 0.0)
            t_out.append(t_out_g)
            nc.vector.memset(t_smb[g][:], 0.0)
            nc.vector.memset(t_stT[g][0:V, :], 0.0)
            nc.sync.dma_start(t_stT[g][V:V + 1, :], onesrow[:, g * BG:(g + 1) * BG])

        # ---------------- preamble: UaH' ----------------
        NSL, SLB = 4, 4
        for ec in range(CH):
            for i in range(NSL):
                b0 = i * SLB
                ps = prep.tile([NC_, T * SLB], f32, tag="rep",
                               name=f"preu{ec}_{i}")
                psv = ps[:, 0:T * SLB].rearrange("p (t b) -> p t b", t=T)
                for dc in range(CH):
                    xs = t_x[:, dc, :, :]
                    x_sl = AP(xs.tensor, xs.offset + b0 * T,
                              [list(xs.ap[0]), [1, T], [T, SLB]])
                    nc.tensor.matmul(psv, t_ua[:, dc, ec, :], x_sl,
                                     start=(dc == 0), stop=(dc == CH - 1))
                dst = t_uahp[:, ec, :, b0:b0 + SLB]
                if i % 2 == 0:
                    nc.scalar.activation(dst, psv, Act.Identity,
                                         bias=t_ba12[:, ec:ec + 1], scale=1.0)
                else:
                    nc.vector.tensor_scalar(dst, psv, t_ba12[:, ec:ec + 1],
                                            None, Op.add)

        # ---------------- preamble: XKC ----------------
        for b in range(BL):
            ps = pxm.tile([T, 113], f32, tag="xmbt", name=f"prex{b}")
            for dc in range(CH):
                nc.tensor.matmul(ps[:, 0:112], t_x[:, dc, b, :], t_w2[:, dc, :],
                                 start=(dc == 0), stop=(dc == CH - 1))
            if b % 2 == 0:
                nc.scalar.activation(t_xkc[:, b, 0:112], ps[:, 0:112],
                                     Act.Identity)
            else:
                nc.vector.tensor_copy(t_xkc[:, b, 0:112], ps[:, 0:112])
        ones_col = AP(t_xkc.tensor, t_xkc[:].offset + 112,
                      [list(t_xkc[:].ap[0]), [113, BL]])
        nc.vector.memset(ones_col, 1.0)

        P = {}

        def emit_bt(g, s):
            """hm/uo matmuls into the xmbt psum tile; reads stateT(s).
            hm is copied to SBUF (DVE may read only one PSUM input, and zr
            already reads xm from PSUM)."""
            xmbt = pxm.tile([BG, 240], f32, tag="xmbt", name=f"xmbt{g}_{s}")
            P[(g, s, "xmbt")] = xmbt
            nc.tensor.matmul(xmbt[:, 128:212], t_stT[g][:], t_wrec[:],
                             start=True, stop=True)
            nc.tensor.matmul(xmbt[:, 212:240], t_stT[g][:], t_uo[:],
                             start=True, stop=True)

        def phase_TH(g, s, half):
            b0 = half * BH
            if s == 0:
                src_ = t_uahp[:, :, :, g * BG + b0:g * BG + b0 + BH]
            else:
                src_ = P[(g, s, "Y")][:, :, :, b0:b0 + BH]
            nc.scalar.activation(t_tanh[g][:, :, :, b0:b0 + BH], src_, Act.Tanh)

        def phase_SC1(g, s):
            """scoresT columns via per-b stationary matmuls (after tanhs)."""
            tg = t_tanh[g]
            scp = psc.tile([T, BG], f32, tag="scp", name=f"scp{g}_{s}")
            P[(g, s, "scp")] = scp
            for b in range(BG):
                for c in range(CH):
                    tc_ = tg[:, c, :, :]
                    stat = AP(tc_.tensor, tc_.offset + b,
                              [list(tc_.ap[0]), [BG, T]])
                    nc.tensor.matmul(scp[:, b:b + 1], stat,
                                     t_va[:, c:c + 1],
                                     start=(c == 0), stop=(c == CH - 1))

        def phase_SC2(g, s):
            """exp -> smb diag; xm accumulation; rZ; xh_n precompute."""
            xmbt_ = P[(g, s, "xmbt")]
            hm = wrk.tile([BG, 84], f32, tag=f"hm{g}", name=f"hm{g}_{s}")
            nc.vector.tensor_copy(hm[:], xmbt_[:, 128:212])
            P[(g, s, "hm")] = hm
            scp = P.pop((g, s, "scp"))
            smb = t_smb[g]
            smb_diag = AP(smb.tensor, smb[:].offset,
                          [list(smb[:].ap[0]), [BG + 1, BG]])
            nc.scalar.activation(smb_diag, scp[:], Act.Exp)
            xm = P[(g, s, "xmbt")][:, 0:113]
            for b in range(BG):
                nc.tensor.matmul(xm, smb[:, BG * b:BG * b + BG],
                                 t_xkc[:, g * BG + b, :],
                                 start=(b == 0), stop=(b == BG - 1))
            rZ = wrk.tile([BG, 1], f32, tag=f"rZ{g}", name=f"rZ{g}_{s}")
            nc.vector.reciprocal(rZ[:], xm[:, 112:113])
            P[(g, s, "rZ")] = rZ
            xh = wrk.tile([BG, V], f32, tag=f"xh{g}", name=f"xh{g}_{s}")
            nc.vector.tensor_scalar(xh[:], xm[:, 56:84], rZ[:], None, Op.mult)
            P[(g, s, "xh")] = xh

        def phase_G1(g, s):
            """zr -> tz; transpose; z1/w1/m1 (off hh-chain)."""
            xmbt = P[(g, s, "xmbt")]
            zr = wrk.tile([BG, 56], f32, tag=f"zr{g}", name=f"zr{g}_{s}")
            nc.vector.scalar_tensor_tensor(zr[:], xmbt[:, 0:56],
                                           P[(g, s, "rZ")][:],
                                           P[(g, s, "hm")][:, 0:56],
                                           Op.mult, Op.add)
            if gru_b0_nonzero:
                nc.vector.tensor_tensor(zr[:], zr[:], t_b0h[:, 0:56], Op.add)
            tz = wrk.tile([BG, 56], f32, tag=f"tz{g}", name=f"tz{g}_{s}")
            nc.scalar.activation(tz[:], zr[:], Act.Tanh)
            P[(g, s, "tz")] = tz
            wast = pwast.tile([NC_, 48], f32, tag="wast", name=f"wast{g}_{s}")
            P[(g, s, "wast")] = wast
            nc.tensor.transpose(wast[0:V, 32:40], tz[:, 0:V], t_i8[:])
            z1 = wrk.tile([V, BG], f32, tag=f"z1{g}", name=f"z1{g}_{s}")
            nc.vector.tensor_scalar(z1[:], wast[0:V, 32:40], 0.5, 0.5,
                                    Op.mult, Op.add)
            w1 = wrk.tile([V, BG], f32, tag=f"w1{g}", name=f"w1{g}_{s}")
            nc.gpsimd.tensor_tensor(w1[:], t_onesT[:], z1[:], Op.subtract)
            m1 = wrk.tile([V, BG], f32, tag=f"m1{g}", name=f"m1{g}_{s}")
            nc.gpsimd.tensor_mul(m1[:], z1[:], t_stT[g][0:V, :])
            P[(g, s, "w1")] = w1
            P[(g, s, "m1")] = m1

        def phase_E1(g, s):
            """output logits + expP (slots between tz_z and hh on ACT)."""
            xmbt = P[(g, s, "xmbt")]
            rZ = P[(g, s, "rZ")]
            l1 = wrk.tile([BG, V], f32, tag=f"l1{g}", name=f"l1{g}_{s}")
            if s > 0:
                l2 = wrk.tile([BG, V], f32, tag=f"l2{g}", name=f"l2{g}_{s}")
                nc.vector.scalar_tensor_tensor(l2[:], t_out[g][:, s - 1, :],
                                               1.0, t_dw[:], Op.is_ge,
                                               Op.mult)
                nc.vector.scalar_tensor_tensor(l1[:], xmbt[:, 84:112], rZ[:],
                                               l2[:], Op.mult, Op.add)
            else:
                nc.vector.tensor_scalar(l1[:], xmbt[:, 84:112], rZ[:], None,
                                        Op.mult)
            logits = wrk.tile([BG, V], f32, tag=f"lg{g}", name=f"lg{g}_{s}")
            nc.vector.tensor_tensor(logits[:], l1[:], xmbt[:, 212:240], Op.add)
            expP = wrk.tile([BG, V], f32, tag=f"eP{g}", name=f"eP{g}_{s}")
            nc.scalar.activation(expP[:], logits[:], Act.Exp)
            P[(g, s, "expP")] = expP

        def phase_G2a(g, s):
            """s1 -> ah -> hh (the on-chain ACT op)."""
            xmbt = P[(g, s, "xmbt")]
            tz = P[(g, s, "tz")]
            s1 = wrk.tile([BG, V], f32, tag=f"s1{g}", name=f"s1{g}_{s}")
            nc.vector.scalar_tensor_tensor(s1[:], tz[:, V:56], 1.0,
                                           P[(g, s, "hm")][:, 56:84],
                                           Op.add, Op.mult)
            ah = wrk.tile([BG, V], f32, tag=f"ah{g}", name=f"ah{g}_{s}")
            nc.vector.tensor_tensor(ah[:], P[(g, s, "xh")][:], s1[:], Op.add)
            if gru_b0_nonzero:
                nc.vector.tensor_tensor(ah[:], ah[:], t_b0h[:, 56:84], Op.add)
            hh = wrk.tile([BG, V], f32, tag=f"hh{g}", name=f"hh{g}_{s}")
            nc.scalar.activation(hh[:], ah[:], Act.Tanh)
            P[(g, s, "hh")] = hh

        def phase_G2b(g, s):
            """state update + WaS/bias for s+1 (hidden under A's tanh2)."""
            wast = P[(g, s, "wast")]
            hh = P[(g, s, "hh")]
            nc.tensor.transpose(wast[0:V, 40:48], hh[:], t_i8[:])
            n1 = wrk.tile([V, BG], f32, tag=f"n1{g}", name=f"n1{g}_{s}")
            nc.vector.tensor_tensor(n1[:], P[(g, s, "w1")][:],
                                    wast[0:V, 40:48], Op.mult)
            nc.vector.tensor_tensor(t_stT[g][0:V, :], n1[:],
                                    P[(g, s, "m1")][:], Op.add)
            if s + 1 >= steps:
                return
            for c in range(CH):
                nc.tensor.matmul(wast[:, c * BG:(c + 1) * BG],
                                 t_wa[:, c * 128:(c + 1) * 128],
                                 t_stT[g][0:V, :], start=True, stop=True)
            emit_bt(g, s + 1)
            wb = t_wasb[g]
            nc.vector.tensor_copy(wb[:].rearrange("p c b -> p (c b)"),
                                  wast[:, 0:CH * BG])
            ty = P[(g, s + 1, "Y")] = wrk.tile([NC_, CH, T, BG], bf16,
                                               tag=f"Y{g}", name=f"Y{g}_{s+1}")
            uah_g = t_uahp[:, :, :, g * BG:(g + 1) * BG]
            def bias(eng, b0, bn):
                w_sl = AP(wb.tensor, wb[:].offset + b0,
                          [list(wb[:].ap[0]), [BG, CH], [0, T], [1, bn]])
                eng.tensor_tensor(ty[:, :, :, b0:b0 + bn],
                                  uah_g[:, :, :, b0:b0 + bn], w_sl, Op.add)
            bias(nc.vector, 0, BH)
            bias(nc.gpsimd, BH, BH)

        def phase_E2(g, s):
            """output softmax normalization (fully off-chain)."""
            xmbt = P.pop((g, s, "xmbt"))
            expP = P.pop((g, s, "expP"))
            for k in ("xh", "tz", "Y", "wast", "w1", "m1", "hh", "rZ", "hm"):
                P.pop((g, s, k), None)
            zp = wrk.tile([BG, 1], f32, tag=f"zp{g}", name=f"zp{g}_{s}")
            nc.vector.tensor_reduce(zp[:], expP[:], mybir.AxisListType.X,
                                    Op.add)
            rp = wrk.tile([BG, 1], f32, tag=f"rp{g}", name=f"rp{g}_{s}")
            nc.vector.reciprocal(rp[:], zp[:])
            rp_bc = AP(rp.tensor, rp[:].offset,
                       [list(rp[:].ap[0]), [0, V]])
            nc.gpsimd.tensor_tensor(t_out[g][:, s, :], expP[:], rp_bc,
                                    Op.mult)

        emit_bt(0, 0)
        emit_bt(1, 0)

        # ---- steps: per half-period, A tanh-ing step s, B walking its
        # attention/gates for the step whose tanhs ran last half ----
        def half(A, s_a, Bs):
            if Bs is not None:
                Bg, s_b = Bs
                phase_SC2(Bg, s_b)
            phase_TH(A, s_a, 0)
            if Bs is not None:
                phase_G1(Bg, s_b)
                phase_G2a(Bg, s_b)
                phase_G2b(Bg, s_b)
                phase_E1(Bg, s_b)
            phase_TH(A, s_a, 1)
            phase_SC1(A, s_a)
            if Bs is not None:
                phase_E2(Bg, s_b)

        for s in range(steps):
            half(0, s, (1, s - 1) if s > 0 else None)
            half(1, s, (0, s))
        phase_SC2(1, steps - 1)
        phase_G1(1, steps - 1)
        phase_E1(1, steps - 1)
        phase_E2(1, steps - 1)

        for g in range(G):
            nc.sync.dma_start(y_out[g * BG:(g + 1) * BG, :, :], t_out[g][:])
    return dr, y_out


_CACHE = {}


def _get_program(gru_b0_nonzero, steps=T):
    key = (bool(gru_b0_nonzero), steps)
    if key in _CACHE:
        return _CACHE[key]
    import concourse.bass as bass
    import concourse.bacc as bacc
    import concourse.tile as tile
    from concourse import mybir

    nc = bacc.Bacc("TRN2", target_bir_lowering=False, debug=False,
                   num_devices=NCORES)
    with tile.TileContext(nc) as tc:
        _build(nc, tc, tile, bass, mybir, gru_b0_nonzero, steps)
    nc.compile()
    _CACHE[key] = nc
    return nc


def _prep_core_inputs(inputs, core):
    import ml_dtypes
    x = inputs["x"]
    xs = np.ascontiguousarray(x[core * BL:(core + 1) * BL]).astype(np.float32)
    x_dmaj = np.ascontiguousarray(
        xs.reshape(BL, T, CH, 128).transpose(3, 2, 0, 1))
    return x_dmaj.astype(ml_dtypes.bfloat16)


def _prep_weights(inputs):
    import ml_dtypes
    f = np.float32
    bfd = ml_dtypes.bfloat16
    Ua = inputs["Ua"].astype(f)
    ua_k = np.ascontiguousarray(
        Ua.reshape(CH, 128, CH, 128).transpose(1, 0, 2, 3))
    ba = (inputs["Ba1"] + inputs["Ba2"]).astype(f).reshape(CH, 128)
    ba12 = np.ascontiguousarray(ba.T)
    Va = inputs["Va"].astype(f).reshape(CH, 128)
    va_t = np.ascontiguousarray(Va.T)
    gk = inputs["gru_kernel"].astype(f).copy()
    gk[:, 0:2 * V] *= 0.5
    w2 = np.concatenate([gk, inputs["Co"]], axis=1).astype(f)
    w2 = np.ascontiguousarray(w2.reshape(CH, 128, 112).transpose(1, 0, 2))
    w = (inputs["emb"].astype(f) @ inputs["Wo"].astype(f)).reshape(-1)
    w0, w1 = float(w[0]), float(w[1])
    gb = inputs["gru_bias"].astype(f)
    out = {
        "ua_k": ua_k.astype(bfd), "ba12": ba12,
        "wa": inputs["Wa"].astype(f),
        "va": va_t.astype(bfd), "w2": w2.astype(bfd),
        "wrec_h": np.concatenate(
            [0.5 * inputs["gru_rec_kernel"].astype(f), 0.5 * gb[1:2]], axis=0),
        "uo": np.concatenate(
            [inputs["Uo"].astype(f), inputs["Bo"].astype(f) + w0], axis=0),
        "i8": np.eye(BG, dtype=f),
        "onesrow": np.ones([1, BL], dtype=f),
        "dwrep": np.full([BG, V], w1 - w0, dtype=f),
    }
    b0 = gb[0].copy()
    b0[0:2 * V] *= 0.5
    if np.any(b0 != 0):
        out["b0rep"] = np.repeat(b0[None, :], BG, axis=0)
    return out, bool(np.any(b0 != 0))


def kernel(**inputs):
    from concourse.bass_utils import run_bass_kernel_spmd

    weights, b0nz = _prep_weights(inputs)
    nc = _get_program(b0nz)
    in_maps = []
    for core in range(NCORES):
        m = dict(weights)
        m["x_b"] = _prep_core_inputs(inputs, core)
        in_maps.append(m)
    res = run_bass_kernel_spmd(nc, in_maps, core_ids=list(range(NCORES)))
    out = np.concatenate([res.results[c]["y"] for c in range(NCORES)], axis=0)
    return out.astype(np.float32)
